# revision 37
# baseline (speedup 1.0000x reference)
"""Multi-headed self-attention on 8 Trainium2 NeuronCores (Bass/Tile).

Problem: B=8, S=1024, D=1024, H=16 heads (DH=64), fp32 in/out.
    qp = q @ Wq.T + bq ; kp = k @ Wk.T + bk ; vp = v @ Wv.T + bv
    out = softmax(Qh Kh^T / sqrt(DH) + maskbias) Vh   (per head, merged)

Sharding: data-parallel over batch - one batch element per core. Host
pre-transposes inputs/weights (layout only) and casts to bf16 (rel-err
budget 2e-2; bf16 attention lands ~2e-3).

Per-core design (v2, bf16):
  - All matmuls bf16 (1 elem/cycle streaming like fp32r, but FWL fast
    weight loads engage and tile_position row-tiling works - the fp32r
    quirks that forced zero-padded K tiles don't apply).
  - Scores are 2-way ROW-TILED on the PE: the two heads of a pair
    occupy partition rows 0-63 / 64-127 of the qp/kp pair tiles, so
    tile_position (0,0)/(64,0) runs both heads' K=64 matmuls
    concurrently - halves score PE time vs the zero-padded scheme.
  - Softmax denominator via the ones-column trick: v_aug per head is
    [V (64 cols) | 1], the AV matmul's row 64 accumulates sum(exp).
  - exp runs on the Scalar engine (1 elem/cycle/lane @1.2GHz,
    dtype-independent) and totals ~147us - comparable to total PE
    time. The kernel is organized as a software pipeline over
    (head-pair, q-chunk) blocks: V-projection first, then per pair
    Q-proj/K-proj/scores/exp/AV/transpose, with a FIFO of deferred
    "rest work" (AV, transposes, next pair's projections) emitted
    between score slots so the PE queue never blocks on ACT.
  - Score PSUM tiles are [128, 1024] = [headA-512 | headB-512] per
    k-tile, so one ACT exps both heads and the masked variant gets its
    per-k-tile bias for free.
  - PSUM budget (8 banks): proj 2 + scores 2x2 + AV 1 + transposes 1
    (all four 66-col transpose outputs pack into one [128,264] bank).
"""

import os
import sys

for _p in (
    "/root/.axon_site",
    "/root/.axon_site/_ro/trn_rl_repo",
    "/root/.axon_site/_ro/pypackages",
    "/opt/trn_rl_repo",
):
    if os.path.isdir(_p) and _p not in sys.path:
        sys.path.append(_p)

from collections import deque

import numpy as np
import ml_dtypes

import concourse.bass as bass
import concourse.tile as tile
import concourse.mybir as mybir
from concourse import bacc
from concourse.bass_utils import run_bass_kernel_spmd
from concourse.masks import make_identity

B, S, D, H = 8, 1024, 1024, 16
DH = D // H  # 64
N_CORES = 8
P = 128

F32 = mybir.dt.float32
BF16 = mybir.dt.bfloat16
BF = ml_dtypes.bfloat16


def build_bass(masked=False, has_bias=True, debug=False):
    s, d, h = S, D, H
    dh = d // h                # 64
    kt_n = d // P              # 8 contraction tiles
    ot_n = d // P              # 8 output-feature tiles (= head pairs)
    st_n = s // P              # 8 sequence tiles of 128
    ch = 512                   # moving-dim chunk (PSUM bank)
    ch_n = s // ch             # 2 chunks per sequence
    qb_n = ch // P             # 4 q-blocks per chunk
    n_pair = h // 2            # 8 head pairs
    vaug_w = h * (dh + 1)      # 1040

    nc = bacc.Bacc("TRN2", target_bir_lowering=False, debug=debug,
                   num_devices=N_CORES)

    # inputs in "half-major" layout [128, 8192]:
    #   col = half*4096 + kt*512 + c  <->  element [kt*128 + p, half*512 + c]
    # of the logical [d, s] transposed operand. Two contiguous 1MB DMAs
    # per operand; the first projection group only needs the first halves.
    bw = (kt_n // 2) * s  # 4096, half width
    qB = nc.dram_tensor("qB", (P, 2 * bw), BF16, kind="ExternalInput").ap()
    kB = nc.dram_tensor("kB", (P, 2 * bw), BF16, kind="ExternalInput").ap()
    vB = nc.dram_tensor("vB", (P, 2 * bw), BF16, kind="ExternalInput").ap()
    wqB = nc.dram_tensor("wqB", (P, 2 * bw), BF16, kind="ExternalInput").ap()
    wkB = nc.dram_tensor("wkB", (P, 2 * bw), BF16, kind="ExternalInput").ap()
    wvB = nc.dram_tensor("wvB", (P, 2 * bw), BF16, kind="ExternalInput").ap()
    if has_bias:
        bqB = nc.dram_tensor("bqB", (P, ot_n), F32, kind="ExternalInput").ap()
        bkB = nc.dram_tensor("bkB", (P, ot_n), F32, kind="ExternalInput").ap()
        bvB = nc.dram_tensor("bvB", (P, vaug_w), F32,
                             kind="ExternalInput").ap()
    mb = nc.dram_tensor("mb", (P, st_n), F32, kind="ExternalInput").ap()
    outd = nc.dram_tensor("out", (s, d), F32, kind="ExternalOutput").ap()

    with tile.TileContext(nc) as tc:
        with tc.tile_pool(name="singles", bufs=1) as singles, \
             tc.tile_pool(name="inputs", bufs=1) as inp, \
             tc.tile_pool(name="vaug", bufs=st_n) as vaugp, \
             tc.tile_pool(name="qkp", bufs=4) as qkp, \
             tc.tile_pool(name="expp", bufs=24) as expp, \
             tc.tile_pool(name="otsp", bufs=4) as otsp, \
             tc.tile_pool(name="finp", bufs=12) as finp, \
             tc.tile_pool(name="rcpp", bufs=16) as rcpp, \
             tc.tile_pool(name="ppsum", bufs=2, space="PSUM") as ppsum, \
             tc.tile_pool(name="spsum", bufs=2, space="PSUM") as spsum, \
             tc.tile_pool(name="opsum", bufs=1, space="PSUM") as opsum, \
             tc.tile_pool(name="tpsum", bufs=1, space="PSUM") as tpsum:

            ident = singles.tile([P, P], F32)
            make_identity(nc, ident)
            idb = singles.tile([P, P], BF16)
            nc.vector.tensor_copy(idb, ident)
            if has_bias:
                bq_t = singles.tile([P, ot_n], F32)
                nc.scalar.dma_start(out=bq_t, in_=bqB)
                bk_t = singles.tile([P, ot_n], F32)
                nc.scalar.dma_start(out=bk_t, in_=bkB)
                bv_t = singles.tile([P, vaug_w], F32)
                nc.scalar.dma_start(out=bv_t, in_=bvB)
                bv_g = bv_t.rearrange("p (g c) -> p g c", c=dh + 1)
            else:
                bq_t = bk_t = bv_g = None
            mb_t = singles.tile([P, st_n], F32)
            if masked:
                nc.scalar.dma_start(out=mb_t, in_=mb)

            # ---- inputs: one [128, 8192] tile per operand, 2 DMAs each,
            # data on the sync HWDGE queue, weights on gpsimd SWDGE ----
            def load(dram, tag, eng):
                """Two separate half-tiles per operand: Tile's dependency
                tracking is tile-granular, so consumers of the first half
                must not wait on the second half's DMA."""
                ts = []
                for hf in range(2):
                    t = inp.tile([P, bw], BF16, tag=f"{tag}{hf}",
                                 name=f"{tag}{hf}")
                    eng.dma_start(out=t, in_=dram[:, hf * bw:(hf + 1) * bw])
                    ts.append(t)
                return ts

            # two queues (FIFO per queue = priority), need-ordered: data on
            # sync, weights on gpsimd. HBM (~358 GB/s) is the aggregate cap.
            v_b = load(vB, "v", nc.sync)
            wv_b = load(wvB, "wv", nc.gpsimd)
            q_b = load(qB, "q", nc.sync)
            wq_b = load(wqB, "wq", nc.gpsimd)
            k_b = load(kB, "k", nc.sync)
            wk_b = load(wkB, "wk", nc.gpsimd)

            def islice(halves, kt, c0, w):
                """Slice [128, w] at logical (kt, cols c0:c0+w) of an
                input in half-major layout (w must stay in one half)."""
                hf, off = divmod(c0, ch)
                assert off + w <= ch
                base = kt * ch + off
                return halves[hf][:, base:base + w]

            def vslice(kt, st):
                """v is st-major: half st//4, col = (st%4)*1024 + kt*128."""
                base = (st % 4) * (kt_n * P) + kt * P
                return v_b[st // 4][:, base:base + P]

            # ============ pipeline: FIFO of deferred emissions ============
            # Each entry: (tag, closure). Closures emit instructions when
            # called; Tile resolves cross-engine deps regardless of order,
            # but same-engine program order must respect producer-before-
            # consumer, which the tagged drain rules below guarantee.
            rest = deque()

            def drain(nmax):
                n = 0
                while rest and n < nmax:
                    rest.popleft()[1]()
                    n += 1

            vaug_tiles = []
            oc_n = d // ch  # 2
            gpc = ch // dh  # 8 head-groups per chunk

            def v_proj_items():
                """V projection -> vaug[st] = [vp | 1] per head. oc-major
                so the first half of wv unlocks the first 8 groups."""
                items = []
                for st in range(st_n):
                    va = vaugp.tile([P, vaug_w], BF16, tag="vaug",
                                    name=f"vaug_{st}")
                    vaug_tiles.append(va)
                for oc in range(oc_n):
                    for st in range(st_n):
                        def body(st=st, oc=oc, last=(oc == oc_n - 1)):
                            va_g = vaug_tiles[st].rearrange(
                                "p (g c) -> p g c", c=dh + 1)
                            ps = ppsum.tile([P, ch], F32, tag="ppsum")
                            for kt in range(kt_n):
                                nc.tensor.matmul(
                                    ps,
                                    vslice(kt, st),
                                    islice(wv_b, kt, oc * ch, ch),
                                    start=(kt == 0),
                                    stop=(kt == kt_n - 1),
                                )
                            g0 = oc * gpc
                            if has_bias:
                                nc.vector.tensor_tensor(
                                    out=va_g[:, g0:g0 + gpc, 0:dh],
                                    in0=ps.rearrange("p (g c) -> p g c", c=dh),
                                    in1=bv_g[:, g0:g0 + gpc, 0:dh],
                                    op=mybir.AluOpType.add,
                                )
                            else:
                                nc.vector.tensor_copy(
                                    va_g[:, g0:g0 + gpc, 0:dh],
                                    ps.rearrange("p (g c) -> p g c", c=dh),
                                )
                            if last:
                                nc.vector.memset(va_g[:, :, dh:dh + 1], 1.0)
                        items.append(("v", body))
                return items

            qp_tiles = {}
            kp_tiles = {}

            def proj_items(which, p):
                """Q or K projection closures for pair tile p [128, s].
                The pair tile is allocated (and registered) immediately so
                later score emission can reference it."""
                w_b, x_b, b_t = (wq_b, q_b, bq_t) if which == "q" \
                    else (wk_b, k_b, bk_t)
                po = qkp.tile([P, s], BF16, tag="qkp", name=f"{which}p_{p}")
                (qp_tiles if which == "q" else kp_tiles)[p] = po
                items = []
                for sc in range(ch_n):
                    def body(sc=sc, po=po):
                        ps = ppsum.tile([P, ch], F32, tag="ppsum")
                        for kt in range(kt_n):
                            nc.tensor.matmul(
                                ps,
                                islice(w_b, kt, p * P, P),
                                islice(x_b, kt, sc * ch, ch),
                                start=(kt == 0),
                                stop=(kt == kt_n - 1),
                            )
                        if has_bias:
                            nc.vector.tensor_scalar_add(
                                po[:, sc * ch:(sc + 1) * ch], ps,
                                b_t[:, p:p + 1])
                        else:
                            nc.vector.tensor_copy(
                                po[:, sc * ch:(sc + 1) * ch], ps)
                    items.append((("proj", p), body))
                return items

            def emit_scores_kt(qp, kp, qc, kt):
                """One score slot: row-tiled pair of K=64 matmuls into a
                shared [128, 1024] = [A|B] psum tile, then exp."""
                ps = spsum.tile([P, 2 * ch], F32, tag="spsum")
                for hp in range(2):
                    r0, r1 = hp * dh, hp * dh + dh
                    nc.tensor.matmul(
                        ps[:, hp * ch:(hp + 1) * ch],
                        kp[r0:r1, kt * P:(kt + 1) * P],
                        qp[r0:r1, qc * ch:(qc + 1) * ch],
                        start=True,
                        stop=True,
                    )
                et = expp.tile([P, 2 * ch], BF16, tag="exp")
                if masked:
                    nc.scalar.activation(
                        et, ps, mybir.ActivationFunctionType.Exp,
                        bias=mb_t[:, kt:kt + 1],
                        scale=1.0 / float(np.sqrt(dh)),
                    )
                else:
                    nc.scalar.activation(
                        et, ps, mybir.ActivationFunctionType.Exp,
                        scale=1.0 / float(np.sqrt(dh)),
                    )
                return et

            def rest_items(h2, qc, exps, fins):
                """AV + transpose + normalize + store for block (h2, qc),
                as a list of emission closures (PE filler granularity)."""
                items = []
                tag = ("rest", h2, qc)
                trs = {}

                avps = {}

                def av(hp, k0, k1):
                    hh = 2 * h2 + hp
                    if k0 == 0:
                        avps[hp] = opsum.tile([dh + 1, ch], F32, tag="opsum",
                                              name=f"av_{h2}_{qc}_{hp}")
                    ot_ps = avps[hp]
                    for kt in range(k0, k1):
                        nc.tensor.matmul(
                            ot_ps,
                            vaug_tiles[kt][:, hh * (dh + 1):(hh + 1) * (dh + 1)],
                            exps[kt][:, hp * ch:(hp + 1) * ch],
                            start=(kt == 0),
                            stop=(kt == st_n - 1),
                        )
                    if k1 == st_n:
                        ots = otsp.tile([dh + 1, ch], BF16, tag="ots")
                        nc.vector.tensor_copy(ots, ot_ps)
                        trs[hp] = ots

                def transp(hp):
                    ots = trs[hp]
                    tr = tpsum.tile([P, qb_n * 66], F32, tag="tpsum")
                    for qb in range(qb_n):
                        nc.tensor.matmul(
                            tr[:, qb * 66:(qb + 1) * 66],
                            ots[:, qb * P:(qb + 1) * P],
                            idb[0:dh + 1, 0:66],
                            start=True,
                            stop=True,
                        )
                    hh = 2 * h2 + hp
                    for qb in range(qb_n):
                        rcp = rcpp.tile([P, 1], F32, tag="rcp")
                        nc.vector.reciprocal(
                            rcp, tr[:, qb * 66 + dh:qb * 66 + dh + 1])
                        nc.vector.tensor_scalar_mul(
                            fins[qb][:, hp * dh:(hp + 1) * dh],
                            tr[:, qb * 66:qb * 66 + dh],
                            rcp,
                        )

                def store():
                    # tail stores go out on the (by then idle) scalar queue
                    eng = nc.scalar if h2 == n_pair - 1 else nc.sync
                    for qb in range(qb_n):
                        row0 = qc * ch + qb * P
                        eng.dma_start(
                            out=outd[row0:row0 + P, h2 * P:(h2 + 1) * P],
                            in_=fins[qb],
                        )

                hk = st_n // 2
                items.append((tag, lambda: av(0, 0, hk)))
                items.append((tag, lambda: av(0, hk, st_n)))
                items.append((tag, lambda: transp(0)))
                items.append((tag, lambda: av(1, 0, hk)))
                items.append((tag, lambda: av(1, hk, st_n)))
                items.append((tag, lambda: transp(1)))
                items.append((tag, store))
                return items

            # prologue: V-projection runs first (its data lands first and
            # its 28us of PE covers the q/k/w loads); the last V groups and
            # pair-0 projections seed the FIFO as pair-0 score fillers.
            v_items = v_proj_items()
            for _, it in v_items[:12]:
                it()
            for _, it in proj_items("q", 0):
                it()
            for _, it in proj_items("k", 0):
                it()
            rest.extend(v_items[12:])

            for h2 in range(n_pair):
                # safety: this pair's projection closures must have emitted
                # (they're queued ahead of rest items, so normally have)
                while rest and rest[0][0] == ("proj", h2):
                    rest.popleft()[1]()
                last_pair = h2 == n_pair - 1
                for qc in range(ch_n):
                    qp = qp_tiles[h2]
                    kp = kp_tiles[h2]
                    exps = []
                    fins = [finp.tile([P, P], F32, tag="fin",
                                      name=f"fin_{h2}_{qc}_{qb}")
                            for qb in range(qb_n)]
                    last_blk = last_pair and qc == ch_n - 1
                    for kt in range(st_n):
                        exps.append(emit_scores_kt(qp, kp, qc, kt))
                        # 9 pops per block matches production (7 rest +
                        # 2 proj avg); otherwise the FIFO grows ~3 items
                        # per pair and the backlog serializes at the tail
                        if kt >= 1:
                            drain(2 if kt in (3, 6) else 1)
                    items = rest_items(h2, qc, exps, fins)
                    if last_blk:
                        # epilogue: emit the final block inline so AV
                        # overlaps the trailing exp ACTs
                        for _, it in items:
                            it()
                        drain(len(rest))
                    else:
                        # projections for the next pair go ahead of this
                        # block's AV work: they get popped as fillers
                        # during the next score phase, keeping ACT fed
                        # across the pair boundary
                        if qc == 0 and h2 + 1 < n_pair:
                            rest.extend(proj_items("q", h2 + 1))
                            rest.extend(proj_items("k", h2 + 1))
                        rest.extend(items)
            drain(len(rest))

    return nc


_CACHE = {}


def _get_compiled(masked=False, has_bias=True):
    key = ("nc", masked, has_bias)
    if key not in _CACHE:
        nc = build_bass(masked=masked, has_bias=has_bias)
        nc.compile()
        _CACHE[key] = nc
    return _CACHE[key]


def kernel(q, k, v, mask, Wq, bq, Wk, bk, Wv, bv):
    q = np.asarray(q, dtype=np.float32)
    k = np.asarray(k, dtype=np.float32)
    v = np.asarray(v, dtype=np.float32)
    mask = np.asarray(mask, dtype=np.float32)
    Wq = np.asarray(Wq, dtype=np.float32)
    Wk = np.asarray(Wk, dtype=np.float32)
    Wv = np.asarray(Wv, dtype=np.float32)
    bq = np.asarray(bq, dtype=np.float32)
    bk = np.asarray(bk, dtype=np.float32)
    bv = np.asarray(bv, dtype=np.float32)

    masked = not bool(np.all(mask == 1.0))
    has_bias = bool(np.any(bq) or np.any(bk) or np.any(bv))
    nc = _get_compiled(masked=masked, has_bias=has_bias)

    ot_n = D // P
    st_n = S // P

    def bmaj(aT):
        """[d, s] -> half-major [128, 8192]: col = half*4096 + kt*512 + c."""
        return np.ascontiguousarray(
            aT.reshape(8, P, 2, 512).transpose(1, 2, 0, 3).reshape(P, 8192)
        ).astype(BF)

    def vmaj(aT):
        """[d, s] -> st-major [128, 8192]:
        col = (st//4)*4096 + (st%4)*1024 + kt*128 + c."""
        return np.ascontiguousarray(
            aT.reshape(8, P, 2, 4, P).transpose(1, 2, 3, 0, 4).reshape(P, 8192)
        ).astype(BF)

    wqB = bmaj(Wq.T)
    wkB = bmaj(Wk.T)
    wvB = bmaj(Wv.T)

    in_maps = []
    for b in range(B):
        mbias = (-10000.0 * (1.0 - mask[b])).astype(np.float32)
        m = {
            "qB": bmaj(q[b].T),
            "kB": bmaj(k[b].T),
            "vB": vmaj(v[b].T),
            "wqB": wqB,
            "wkB": wkB,
            "wvB": wvB,
            "mb": np.ascontiguousarray(mbias.reshape(st_n, P).T),
        }
        if has_bias:
            m["bqB"] = np.ascontiguousarray(bq.reshape(ot_n, P).T)
            m["bkB"] = np.ascontiguousarray(bk.reshape(ot_n, P).T)
            bv_aug = np.concatenate(
                [bv.reshape(H, DH), np.ones((H, 1), np.float32)], axis=1
            ).reshape(-1).astype(np.float32)
            m["bvB"] = np.ascontiguousarray(
                np.broadcast_to(bv_aug, (P, H * (DH + 1))))
        in_maps.append(m)

    _CACHE["in_maps"] = in_maps
    _CACHE["last_nc"] = nc
    res = run_bass_kernel_spmd(nc, in_maps, core_ids=list(range(N_CORES)))
    out = np.stack([res.results[b]["out"] for b in range(B)], axis=0)
    return out.astype(np.float32)
